# revision 43
# baseline (speedup 1.0000x reference)
"""GuidedFilter Trainium2 kernel: batch-parallel over 8 NeuronCores.

Per core: img [1,512,512] f32, feat [16,512,512] f32 -> out [16,512,512] f32.

Each 2-D reflect box blur (radius 5) is two TensorE passes, BOTH in
data-as-weights form with tight band windows:
  pass A: T1[w, i] = sum_h X[h, w] * B[i, h]   (lhsT = X column block,
          rhs = G window; psum partitions = w, free = i)
  pass C: out[i, u] = sum_w T1[w, i] * B[u, w] (lhsT = T1 column block,
          rhs = same G; psum partitions = i, free = u)
Layout is preserved: [h-part within 128-block, free=(block, w)] in and out.
Each source block j only touches output rows [128j-5, 128j+132] (reflect at
edges), so each pass streams 542 rows instead of 2048: band-overlap strips
(10 cols at block seams) are written as separate accumulate matmuls.
G is the unnormalized box matrix (entries {0,1,2}, exact in bf16); the
1/121 normalization is folded into the psum-evacuation ops.

Both elementwise subtractions are folded into TensorE via blur linearity,
with the b-side rewritten to decouple its inputs from the a-chain:
  a = t2 - t1            -> blur(a) = dual A-pass of (t2,+G16),(t1,-G16)
  b = mp - a*mI = mp*q - t2*mI  (q = 1 + mIR*mI, shared)
                         -> blur(b) = dual A-pass of (z1,+G16),(z2,-G16)
so a, u2, b never materialize. Dual-blur inputs (t1, t2, z1, z2) are fp16
(same matmul/DVE speed as bf16, 8x finer mantissa — keeps the z1-z2
cancellation benign; measured rel err improves vs the bf16 a/b pipeline).

Engine routing (hardware-legal set: GPSIMD/Pool cannot access PSUM and has
no scalar_tensor_tensor; Act does unary copy/scale only):
  Act: psum->sbuf handoffs (T1) + mp evac;  DVE: psum stt consumers
  (t2/v/o) + z1 = mp*q, z2 = t2*mI;  Pool: Pd = Xd*I and t1 = mp*mIR
  (sbuf tensor_mul, both slack-tolerant) + SWDGE loads (the only DMA
  path that casts f32->bf16); stores are f32->f32 via SP HWDGE.

Blur2 calls are emitted software-pipelined: the C-pass of blur k is emitted
after the A-pass of blur k+2 (PENDING=2), and the per-channel a/b blurs
trail the X/P blurs by LEAD=3 channels, hiding psum handoff latency and the
DVE a-chain from the in-order PE queue. Pd for channel d+2 is emitted at
iteration d so the DVE queue never stalls on a fresh DMA.
"""
import sys

sys.path.insert(0, "/opt/trn_rl_repo")

import numpy as np
import ml_dtypes

RADIUS = 5
EPS = 1e-08
HOFF_STYLE = "halves"   # or "quarters"
HOFF_XP = (("A", "A"), ("A", "A"))
HOFF_AB = (("A", "A"), ("A", "A"))
MP_ENG = ("A", "A")
T2_ENG = ("V", "V")
V_ENG = ("V", "V")
O_ENG = ("V", "V")
U2_ENG = "V"
T1_ENG = "P"
PD_ENG = "P"
SHARED_P = False
T1_EARLY = False
SHARED_P2 = False
AB_FIRST = False
TAPER = False
B_ENG = "P"
PENDING = 2
LEAD = 3
H = W = 512
D = 16
NCORES = 8
U = 1.0 / 121.0  # box normalization (11x11)

# Per source block j: output-row window [lo, hi) touched by its 128 rows.
WIN = [(0, 133), (123, 261), (251, 389), (379, 512)]
GOFF = [0, 133, 271, 409]  # column offset of window j in packed G
GW = 542
# Segments per source block: (out_lo, out_hi, start, stop).
SEGS = [
    [(0, 123, True, True), (123, 133, True, False)],
    [(123, 133, False, True), (133, 251, True, True), (251, 261, True, False)],
    [(251, 261, False, True), (261, 379, True, True), (379, 389, True, False)],
    [(379, 389, False, True), (389, 512, True, True)],
]


def _box_matrix():
    B = np.zeros((512, 512), np.float32)
    for i in range(512):
        for d in range(-RADIUS, RADIUS + 1):
            j = i + d
            if j < 0:
                j = -j
            elif j > 511:
                j = 1022 - j
            B[i, j] += 1.0
    return B


def _g_packed():
    """G [128, 542]: G[p, GOFF[j]+c] = B[WIN[j][0]+c, 128j+p]."""
    B = _box_matrix()
    cols = []
    for j in range(4):
        lo, hi = WIN[j]
        cols.append(B[lo:hi, 128 * j:128 * (j + 1)].T)
    return np.ascontiguousarray(np.concatenate(cols, axis=1)).astype(
        ml_dtypes.bfloat16)


def _build_bass():
    import concourse.bass as bass
    import concourse.bacc as bacc
    import concourse.tile as tile
    from concourse import mybir

    f32 = mybir.dt.float32
    bf16 = mybir.dt.bfloat16
    Alu = mybir.AluOpType
    Act = mybir.ActivationFunctionType

    nc = bacc.Bacc("TRN2", target_bir_lowering=False, debug=False,
                   num_devices=NCORES)

    feat_d = nc.dram_tensor("feat", [D, H, W], f32, kind="ExternalInput").ap()
    img_d = nc.dram_tensor("img", [1, H, W], f32, kind="ExternalInput").ap()
    g_d = nc.dram_tensor("gmat", [128, GW], bf16, kind="ExternalInput").ap()
    gn_d = nc.dram_tensor("gneg", [128, GW], bf16, kind="ExternalInput").ap()
    out_d = nc.dram_tensor("out", [D, H, W], f32, kind="ExternalOutput").ap()

    def ld(dst, src2d):
        # HBM [512,512] f32 -> SBUF [128, (j,w)] bf16 (SWDGE: casts)
        nc.gpsimd.dma_start(
            out=dst.rearrange("p (j w) -> p j w", j=4),
            in_=src2d.rearrange("(j p) w -> p j w", p=128))

    with tile.TileContext(nc) as tc:
        with (
            tc.tile_pool(name="consts", bufs=1) as consts,
            tc.tile_pool(name="shared", bufs=1) as shared,
            tc.tile_pool(name="chan", bufs=2) as chan,
            tc.tile_pool(name="psum", bufs=1, space="PSUM") as psum,
        ):
            G = consts.tile([128, GW], bf16)
            nc.sync.dma_start(out=G[:], in_=g_d)
            Gn = consts.tile([128, GW], bf16)
            nc.sync.dma_start(out=Gn[:], in_=gn_d)
            fp16 = mybir.dt.float16
            G16 = consts.tile([128, GW], fp16)
            nc.vector.tensor_copy(G16[:], G[:])
            Gn16 = consts.tile([128, GW], fp16)
            nc.vector.tensor_copy(Gn16[:], Gn[:])
            I = consts.tile([128, 2048], bf16)
            ld(I, img_d[0])

            def copy_half(eng, dst, src, scale=None):
                if eng == "A":
                    if scale is None:
                        nc.scalar.copy(dst, src)
                    else:
                        nc.scalar.activation(dst, src, Act.Copy, 0.0, scale)
                elif eng == "V":
                    if scale is None:
                        nc.vector.tensor_copy(dst, src)
                    else:
                        nc.vector.tensor_scalar_mul(dst, src, scale)
                else:
                    if scale is None:
                        nc.gpsimd.tensor_copy(dst, src)
                    else:
                        nc.gpsimd.tensor_scalar_mul(dst, src, scale)

            def stt(eng, dst, ps, s, t, op0, op1):
                e = nc.vector if eng == "V" else nc.gpsimd
                e.scalar_tensor_tensor(dst, ps, s, t, op0=op0, op1=op1)

            def emit_pass(ph, inputs):
                """ph: [tileA(1024), tileB(1024)]; out tile t -> ph[t//2].
                inputs: list of (lhsT_fn, Gtile) accumulated into the same
                psum regions (linear combination folded into TensorE)."""
                last = len(inputs) - 1
                for t in range(4):
                    pst, base = ph[t // 2], 512 * (t % 2)
                    for j in range(4):
                        lo0 = WIN[j][0]
                        for (lo, hi, st, sp) in SEGS[j]:
                            for idx, (lhsT_fn, Gt) in enumerate(inputs):
                                nc.tensor.matmul(
                                    pst[:, base + lo:base + hi],
                                    lhsT_fn(t, j),
                                    Gt[:, GOFF[j] + lo - lo0:
                                       GOFF[j] + hi - lo0],
                                    start=(st if idx == 0 else False),
                                    stop=(sp if idx == last else False),
                                    skip_group_check=True)

            # ---- software-pipelined blur emission --------------------------
            # Each entry: dict with X (input tile), hoff engines, and a
            # consume(C01, C23) callback emitted right after its C-pass.
            pending = []  # at most 1 deferred C-pass

            def emit_blur_A(Xs, hoff, consume):
                # Xs: list of (tile, Gtile) accumulated as sum_k G_k-blur(X_k)
                A01 = psum.tile([128, 1024], f32, tag="A01")
                A23 = psum.tile([128, 1024], f32, tag="A23")
                emit_pass([A01, A23], [
                    ((lambda Xk: (lambda wb, j: Xk[
                        :, 512 * j + 128 * wb:512 * j + 128 * (wb + 1)]))(Xk),
                     Gk) for (Xk, Gk) in Xs])
                T1 = chan.tile([128, 2048], bf16, tag="T1", bufs=4)
                if HOFF_STYLE == "quarters":
                    qe = ("A", "V", "A", "P")
                    for q in range(4):
                        src_t = (A01, A01, A23, A23)[q]
                        lo = 512 * (q % 2)
                        copy_half(qe[q], T1[:, 512 * q:512 * (q + 1)],
                                  src_t[:, lo:lo + 512])
                elif HOFF_STYLE == "asym":
                    # small head [512] then [512]+[1024]: finer A01 release
                    copy_half(hoff[0], T1[:, 0:512], A01[:, 0:512])
                    copy_half(hoff[0], T1[:, 512:1024], A01[:, 512:1024])
                    copy_half(hoff[1], T1[:, 1024:2048], A23[:])
                else:
                    copy_half(hoff[0], T1[:, 0:1024], A01[:])
                    copy_half(hoff[1], T1[:, 1024:2048], A23[:])
                pending.append((T1, consume))

            def flush_C():
                if not pending:
                    return
                T1, consume = pending.pop(0)
                C01 = psum.tile([128, 1024], f32, tag="C01")
                C23 = psum.tile([128, 1024], f32, tag="C23")
                emit_pass([C01, C23], [(lambda ib, wb: T1[
                    :, 512 * wb + 128 * ib:512 * wb + 128 * (ib + 1)], G)])
                consume(C01, C23)

            def blur(X, hoff, consume):
                if not isinstance(X, list):
                    X = [(X, G)]
                emit_blur_A(X, hoff, consume)
                # keep two A->C gaps: hides handoff latency from in-order PE
                if len(pending) > PENDING:
                    flush_C()

            # ---- shared (img) stage ---------------------------------------
            I2 = shared.tile([128, 2048], bf16)
            q = shared.tile([128, 2048], mybir.dt.float16)
            mIs = shared.tile([128, 2048], bf16)
            mIR = shared.tile([128, 2048], bf16)
            R = shared.tile([128, 2048], f32)
            m2 = shared.tile([128, 2048], f32)
            vps = shared.tile([128, 2048], f32)

            if SHARED_P or SHARED_P2:
                nc.gpsimd.tensor_mul(I2[:], I[:], I[:])
            else:
                nc.vector.tensor_mul(I2[:], I[:], I[:])

            def consume_I(C01, C23):
                copy_half("A", mIs[:, 0:1024], C01[:], scale=U)
                copy_half("A", mIs[:, 1024:2048], C23[:], scale=U)
                if SHARED_P or SHARED_P2:
                    nc.gpsimd.tensor_mul(m2[:], mIs[:], mIs[:])
                else:
                    nc.vector.tensor_mul(m2[:], mIs[:], mIs[:])

            def consume_I2(C01, C23):
                # vps = U*corrI_raw - mI^2 + EPS ; R = 1/vps ; mIR = mI*R
                stt("V", vps[:, 0:1024], C01[:], U, m2[:, 0:1024],
                    Alu.mult, Alu.subtract)
                stt("V", vps[:, 1024:2048], C23[:], U, m2[:, 1024:2048],
                    Alu.mult, Alu.subtract)
                if SHARED_P:
                    nc.gpsimd.tensor_scalar_add(vps[:], vps[:], EPS)
                else:
                    nc.vector.tensor_scalar_add(vps[:], vps[:], EPS)
                nc.vector.reciprocal_approx_fast(R[:], vps[:])
                if SHARED_P:
                    nc.gpsimd.tensor_mul(mIR[:], mIs[:], R[:])
                else:
                    nc.vector.tensor_mul(mIR[:], mIs[:], R[:])
                nc.vector.tensor_mul(q[:], mIR[:], mIs[:])
                nc.vector.tensor_scalar_add(q[:], q[:], 1.0)

            blur(I, ("A", "A"), consume_I)
            blur(I2, ("A", "A"), consume_I2)

            # ---- per-channel state ----------------------------------------
            ch_state = {}

            def load_pair(d0):
                # one SWDGE DMA for channels d0, d0+1 (halves fixed cost)
                X2 = chan.tile([128, 4096], bf16, tag="xd", bufs=3)
                nc.gpsimd.dma_start(
                    out=X2.rearrange("p (c j w) -> p c j w", c=2, j=4),
                    in_=feat_d[d0:d0 + 2].rearrange(
                        "c (j p) w -> p c j w", p=128))
                for k in range(2):
                    ch_state[d0 + k] = {"Xd": X2[:, 2048 * k:2048 * (k + 1)]}

            def make_Pd(d):
                # emitted well after the DMA so the DVE queue never stalls
                # on a fresh load; fills DVE's wait-for-C(X) bubble
                s = ch_state[d]
                Pd = chan.tile([128, 2048], bf16, tag="pd", bufs=4)
                if PD_ENG == "V":
                    nc.vector.tensor_mul(Pd[:], s["Xd"], I[:])
                else:
                    nc.gpsimd.tensor_mul(Pd[:], s["Xd"], I[:])
                s["Pd"] = Pd

            def emit_XP(d):
                s = ch_state[d]
                mp = chan.tile([128, 2048], bf16, tag="mp", bufs=2)
                s["mp"] = mp

                def consume_X(C01, C23):
                    fp16 = mybir.dt.float16
                    copy_half(MP_ENG[0], mp[:, 0:1024], C01[:], scale=U)
                    copy_half(MP_ENG[1], mp[:, 1024:2048], C23[:], scale=U)
                    t1 = chan.tile([128, 2048], fp16, tag="t1m", bufs=4)
                    if T1_ENG == "V":
                        nc.vector.tensor_mul(t1[:], mp[:], mIR[:])
                    else:
                        nc.gpsimd.tensor_mul(t1[:], mp[:], mIR[:])
                    z1 = chan.tile([128, 2048], fp16, tag="z1", bufs=4)
                    nc.vector.tensor_mul(z1[:], mp[:], q[:])
                    s["t1"] = t1
                    s["z1"] = z1

                def consume_P(C01, C23):
                    fp16 = mybir.dt.float16
                    t2 = chan.tile([128, 2048], fp16, tag="t2", bufs=4)
                    stt(T2_ENG[0], t2[:, 0:1024], C01[:], U, R[:, 0:1024],
                        Alu.mult, Alu.mult)
                    stt(T2_ENG[1], t2[:, 1024:2048], C23[:], U,
                        R[:, 1024:2048], Alu.mult, Alu.mult)
                    z2 = chan.tile([128, 2048], fp16, tag="z2", bufs=4)
                    nc.vector.tensor_mul(z2[:], t2[:], mIs[:])
                    s["t2"] = t2
                    s["z2"] = z2

                blur(s["Xd"], HOFF_XP[0], consume_X)
                blur(s["Pd"], HOFF_XP[1], consume_P)

            def emit_ab(d):
                s = ch_state[d]
                v = chan.tile([128, 2048], bf16, tag="v", bufs=2)
                o = chan.tile([128, 2048], f32, tag="o", bufs=2)

                def consume_a(C01, C23):
                    stt(V_ENG[0], v[:, 0:1024], C01[:], U, I[:, 0:1024],
                        Alu.mult, Alu.mult)
                    stt(V_ENG[1], v[:, 1024:2048], C23[:], U, I[:, 1024:2048],
                        Alu.mult, Alu.mult)

                def consume_b(C01, C23):
                    stt(O_ENG[0], o[:, 0:1024], C01[:], U, v[:, 0:1024],
                        Alu.mult, Alu.add)
                    stt(O_ENG[1], o[:, 1024:2048], C23[:], U, v[:, 1024:2048],
                        Alu.mult, Alu.add)
                    nc.sync.dma_start(
                        out=out_d[d].rearrange("(j p) w -> p j w", p=128),
                        in_=o.rearrange("p (j w) -> p j w", j=4))

                blur([(s["t2"], G16), (s["t1"], Gn16)], HOFF_AB[0],
                     consume_a)
                blur([(s["z1"], G16), (s["z2"], Gn16)], HOFF_AB[1],
                     consume_b)
                del ch_state[d]

            # channel schedule: X/P run ~LEAD channels ahead of a/b
            load_pair(0)
            load_pair(2)
            if LEAD >= 4:
                load_pair(4)
            make_Pd(0)
            make_Pd(1)
            ab_next = [0]
            for d in range(D + LEAD):
                if d >= D and ab_next[0] >= D:
                    break
                if d + 2 < D:
                    make_Pd(d + 2)
                if AB_FIRST:
                    if d >= LEAD:
                        emit_ab(d - LEAD)
                    if d < D:
                        emit_XP(d)
                        dn = d + LEAD + 1
                        if dn < D and dn % 2 == 0:
                            load_pair(dn)
                else:
                    if d < D:
                        emit_XP(d)
                        dn = d + LEAD + 1
                        if dn < D and dn % 2 == 0:
                            load_pair(dn)
                    if d >= LEAD:
                        emit_ab(ab_next[0])
                        ab_next[0] += 1
                    if TAPER and D - LEAD <= d < D and ab_next[0] < D:
                        emit_ab(ab_next[0])
                        ab_next[0] += 1
            while pending:
                flush_C()

    nc.compile()
    return nc


_NC_CACHE = None


def kernel(feat: np.ndarray, img: np.ndarray) -> np.ndarray:
    global _NC_CACHE
    from concourse.bass_utils import run_bass_kernel_spmd

    if _NC_CACHE is None:
        _NC_CACHE = _build_bass()
    nc = _NC_CACHE
    g = _g_packed()
    feat = np.asarray(feat, np.float32)
    img = np.asarray(img, np.float32)
    gn = np.negative(g)
    in_maps = [
        {"feat": feat[c], "img": img[c], "gmat": g, "gneg": gn}
        for c in range(NCORES)
    ]
    res = run_bass_kernel_spmd(nc, in_maps, list(range(NCORES)))
    return np.stack([res.results[c]["out"] for c in range(NCORES)], axis=0)
